# revision 1
# baseline (speedup 1.0000x reference)
"""Bahdanau-style attention scores kernel for 8 TRN2 NeuronCores.

Reference math (B=64, S=2048, E=512, D=512):
    Wh = attn_W[:D]; We = attn_W[D:]
    h_proj = hidden @ Wh                                  # [B, D]
    e_proj[b,s,:] = enc[b,s,:] @ We                       # [B, S, D]
    energy = tanh(h_proj[:,None,:] + e_proj + attn_b)     # [B, S, D]
    scores = energy @ v_w                                 # [B, S]
    out = softmax(scores, axis=1)

Sharding: data-parallel over batch, 8 batches per core.
Host precomputes c = hidden @ Wh + attn_b (tiny: 33 MFLOP) and the
encoder transpose to [b, e, s] layout so the contraction axis e lands on
SBUF partitions.

Per-core kernel (b = 0..7 local batches):
  e_projT[d, s] = We.T @ encT          (4x4x4 tiled matmul, f32r)
  energyT = tanh(e_projT + c_b[d])     (ACT, bias per-partition)
  scores[s] = v.T @ energyT            (matvec accumulated over d chunks)
  out = exp(scores) / sum(exp)         (ACT exp + accum, DVE normalize)
"""

import numpy as np

import concourse.bass as bass  # noqa: F401  (engine namespaces via nc)
import concourse.mybir as mybir
import concourse.tile as tile
from concourse import bacc, bass_isa
from concourse.bass_utils import run_bass_kernel_spmd

N_CORES = 8
B, S, E, D = 64, 2048, 512, 512
BL = B // N_CORES          # local batches per core
P = 128                    # partition tile
EC = E // P                # e chunks (4)
DC = D // P                # d chunks (4)
ST = 512                   # s tile (free dim per matmul)
SC = S // ST               # s tiles (4)

MM_DT = mybir.dt.float32r  # matmul compute dtype (f32 data, full-rate PE)

_COMPILED = None  # nc cache within the process


def _build(loop_reps=None, enc_bufs=2, mm_dt=None, sgroup=1, psp_bufs=None,
           scp_bufs=2, en_bufs=4, mv_delay=1, pack=False):
    global MM_DT
    if mm_dt is not None:
        MM_DT = mm_dt
    if psp_bufs is None:
        psp_bufs = {1: 4, 2: 3}[sgroup]
    if pack:
        psp_bufs, scp_bufs, en_bufs = 7, 1, 6
    MV_DT = mybir.dt.bfloat16 if pack else MM_DT
    nc = bacc.Bacc(
        "TRN2", target_bir_lowering=False, debug=False, num_devices=N_CORES
    )
    f32 = mybir.dt.float32

    enc_ap = nc.dram_tensor("enc_t", [BL, E, S], MM_DT, kind="ExternalInput").ap()
    we_ap = nc.dram_tensor("we", [E, D], MM_DT, kind="ExternalInput").ap()
    c_ap = nc.dram_tensor("c", [P, BL * DC], f32, kind="ExternalInput").ap()
    v_ap = nc.dram_tensor("v", [P, DC], MV_DT, kind="ExternalInput").ap()
    out_ap = nc.dram_tensor("out", [BL, S], f32, kind="ExternalOutput").ap()

    with tile.TileContext(nc) as tc:
        with (
            tc.tile_pool(name="singles", bufs=1) as singles,
            tc.tile_pool(name="encp", bufs=enc_bufs) as encp,
            tc.tile_pool(name="enp", bufs=en_bufs) as enp,
            tc.tile_pool(name="expp", bufs=2) as expp,
            tc.tile_pool(name="smallp", bufs=2) as smallp,
            tc.tile_pool(name="outp", bufs=2) as outp,
            tc.tile_pool(name="psp", bufs=psp_bufs, space="PSUM") as psp,
            tc.tile_pool(name="scp", bufs=scp_bufs, space="PSUM") as scp,
        ):
            we_sb = singles.tile([P, EC, D], MM_DT)
            nc.sync.dma_start(
                out=we_sb, in_=we_ap.rearrange("(e p) d -> p e d", p=P)
            )
            c_sb = singles.tile([P, BL * DC], f32)
            v_sb = singles.tile([P, DC], MV_DT)

            def emit_cv_dmas():
                nc.sync.dma_start(out=c_sb, in_=c_ap)
                nc.sync.dma_start(out=v_sb, in_=v_ap)

            def emit_batches():
                for b in range(BL):
                    enc_sb = encp.tile([P, EC, S], MM_DT, name=f"enc_b{b}", tag="enc")
                    enc_src = enc_ap[b].rearrange("(e p) s -> p e s", p=P)
                    # split by s-block so matmuls can start after the first
                    # 1MB lands instead of waiting for the whole 4MB batch
                    for s in range(SC):
                        nc.sync.dma_start(
                            out=enc_sb[:, :, s * ST : (s + 1) * ST],
                            in_=enc_src[:, :, s * ST : (s + 1) * ST],
                        )
                        if b == 0 and s == 0:
                            emit_cv_dmas()
                    exp_sb = expp.tile([1, S], f32, name=f"exp_b{b}", tag="exp")
                    sums = smallp.tile([1, SC], f32, name=f"sums_b{b}", tag="sums")

                    pend = []

                    def emit_mv(sc_ps, s, d, en_t, exp_t, sums_t):
                        nc.tensor.matmul(
                            sc_ps,
                            lhsT=v_sb[:, d : d + 1],
                            rhs=en_t,
                            start=(d == 0),
                            stop=(d == DC - 1),
                        )
                        if d == DC - 1:
                            nc.scalar.activation(
                                exp_t[:, s * ST : (s + 1) * ST],
                                sc_ps,
                                mybir.ActivationFunctionType.Exp,
                                accum_out=sums_t[:, s : s + 1],
                            )

                    G = sgroup
                    sc_tiles = {}
                    for sg in range(SC // G):
                        for d in range(DC):
                            ps = psp.tile(
                                [P, G * ST], f32, name=f"ps_b{b}g{sg}d{d}", tag="ps"
                            )
                            for g in range(G):
                                s = sg * G + g
                                for e in range(EC):
                                    nc.tensor.matmul(
                                        ps[:, g * ST : (g + 1) * ST],
                                        lhsT=we_sb[:, e, d * P : (d + 1) * P],
                                        rhs=enc_sb[:, e, s * ST : (s + 1) * ST],
                                        start=(e == 0),
                                        stop=(e == EC - 1),
                                    )
                            en = enp.tile(
                                [P, G * ST], MM_DT, name=f"en_b{b}g{sg}d{d}",
                                tag="en",
                            )
                            nc.scalar.activation(
                                en,
                                ps,
                                mybir.ActivationFunctionType.Tanh,
                                bias=c_sb[:, b * DC + d : b * DC + d + 1],
                            )
                            for g in range(G):
                                s = sg * G + g
                                if s not in sc_tiles:
                                    sc_tiles[s] = scp.tile(
                                        [1, ST], f32, name=f"sc_b{b}s{s}", tag="sc"
                                    )
                                pend.append(
                                    (sc_tiles[s], s, d,
                                     en[:, g * ST : (g + 1) * ST], exp_sb, sums)
                                )
                            while len(pend) > G * mv_delay:
                                emit_mv(*pend.pop(0))
                    while pend:
                        emit_mv(*pend.pop(0))

                    ssum = smallp.tile([1, 1], f32, name=f"ssum_b{b}", tag="ssum")
                    nc.vector.reduce_sum(ssum, sums, axis=mybir.AxisListType.X)
                    rec = smallp.tile([1, 1], f32, name=f"rec_b{b}", tag="rec")
                    nc.vector.reciprocal(rec, ssum)
                    outsb = outp.tile([1, S], f32, name=f"out_b{b}", tag="outsb")
                    nc.vector.tensor_scalar_mul(outsb, exp_sb, rec)
                    nc.sync.dma_start(out=out_ap[b : b + 1, :], in_=outsb)

            def emit_batches_packed():
                # d-outer loop: weights we[e,d] reused across the 4-s sweep;
                # the 4 per-s matvecs of each d issue back-to-back into four
                # PE column groups (tile_position) and execute concurrently,
                # writing score rows at psum partitions 0/32/64/96.
                pend = []   # one quad of deferred matvecs
                tails = []  # deferred per-batch softmax tails

                def flush_pend():
                    for scores_t, s, d, en_t in pend:
                        nc.tensor.matmul(
                            scores_t[32 * s : 32 * s + 1, :],
                            lhsT=v_sb[:, d : d + 1],
                            rhs=en_t,
                            start=(d == 0),
                            stop=(d == DC - 1),
                            tile_position=(0, 32 * s),
                        )
                    pend.clear()

                def flush_and_tail():
                    flush_pend()
                    if tails:
                        tails.pop(0)()

                for b in range(BL):
                    enc_sb = encp.tile([P, EC, S], MM_DT, name=f"enc_b{b}", tag="enc")
                    enc_src = enc_ap[b].rearrange("(e p) s -> p e s", p=P)
                    for s in range(SC):
                        nc.sync.dma_start(
                            out=enc_sb[:, :, s * ST : (s + 1) * ST],
                            in_=enc_src[:, :, s * ST : (s + 1) * ST],
                        )
                        if b == 0 and s == 0:
                            emit_cv_dmas()

                    scores = scp.tile([P, ST], f32, name=f"scores_b{b}", tag="scores")

                    for d in range(DC):
                        ps_tiles = []
                        for s in range(SC):
                            ps = psp.tile(
                                [P, ST], f32, name=f"ps_b{b}d{d}s{s}", tag="ps"
                            )
                            ps_tiles.append((s, ps))
                        for e in range(EC):
                            for s, ps in ps_tiles:
                                nc.tensor.matmul(
                                    ps,
                                    lhsT=we_sb[:, e, d * P : (d + 1) * P],
                                    rhs=enc_sb[:, e, s * ST : (s + 1) * ST],
                                    start=(e == 0),
                                    stop=(e == EC - 1),
                                )
                        flush_and_tail()
                        for s, ps in ps_tiles:
                            en = enp.tile(
                                [P, ST], MV_DT, name=f"en_b{b}d{d}s{s}", tag="en"
                            )
                            nc.scalar.activation(
                                en,
                                ps,
                                mybir.ActivationFunctionType.Tanh,
                                bias=c_sb[:, b * DC + d : b * DC + d + 1],
                            )
                            pend.append((scores, s, d, en))

                    def make_tail(b=b, scores=scores):
                        def tail():
                            ssum4 = smallp.tile(
                                [P, 1], f32, name=f"ssum4_b{b}", tag="ssum4"
                            )
                            nc.vector.memset(ssum4, 0.0)
                            expw = expp.tile(
                                [P, ST], f32, name=f"expw_b{b}", tag="expw"
                            )
                            nc.vector.memset(expw, 0.0)
                            for s in range(SC):
                                nc.scalar.activation(
                                    expw[32 * s : 32 * s + 1, :],
                                    scores[32 * s : 32 * s + 1, :],
                                    mybir.ActivationFunctionType.Exp,
                                    accum_out=ssum4[32 * s : 32 * s + 1, 0:1],
                                )
                            tot = smallp.tile([P, 1], f32, name=f"tot_b{b}", tag="tot")
                            nc.gpsimd.partition_all_reduce(
                                tot, ssum4, 128, bass_isa.ReduceOp.add
                            )
                            rec = smallp.tile([P, 1], f32, name=f"rec_b{b}", tag="rec")
                            nc.vector.reciprocal(rec, tot)
                            outw = outp.tile(
                                [P, ST], f32, name=f"outw_b{b}", tag="outw"
                            )
                            nc.vector.tensor_scalar_mul(outw, expw, rec)
                            nc.sync.dma_start(
                                out=out_ap[b].rearrange("(r s) -> r s", r=SC),
                                in_=outw[0 : 32 * (SC - 1) + 1 : 32, :],
                            )
                        return tail

                    tails.append(make_tail())

                flush_and_tail()
                flush_and_tail()

            emit = emit_batches_packed if pack else emit_batches

            if loop_reps is None:
                emit()
            else:
                with tc.For_i(
                    0,
                    loop_reps,
                    1,
                    hint_engines=(
                        mybir.EngineType.PE,
                        mybir.EngineType.Activation,
                    ),
                ):
                    emit()

    nc.compile()
    return nc


def _get_nc():
    global _COMPILED
    if _COMPILED is None:
        _COMPILED = _build()
    return _COMPILED


def _prep_in_maps(hidden, encoder_outputs, attn_W, attn_b, v_w, bf16=False,
                  v_bf16=False):
    hidden = np.asarray(hidden, dtype=np.float32)
    encoder_outputs = np.asarray(encoder_outputs, dtype=np.float32)
    attn_W = np.asarray(attn_W, dtype=np.float32)
    attn_b = np.asarray(attn_b, dtype=np.float32)
    v_w = np.asarray(v_w, dtype=np.float32)

    c_full = hidden @ attn_W[:D] + attn_b            # [B, D]
    we = np.ascontiguousarray(attn_W[D:])            # [E, D]
    v = np.ascontiguousarray(v_w.reshape(DC, P).T)   # [P, DC]

    in_maps = []
    for i in range(N_CORES):
        lo = i * BL
        enc_t = np.ascontiguousarray(
            encoder_outputs[:, lo : lo + BL, :].transpose(1, 2, 0)
        )                                            # [BL, E, S]
        c_shard = c_full[lo : lo + BL]               # [BL, D]
        c = np.ascontiguousarray(
            c_shard.reshape(BL, DC, P).transpose(2, 0, 1).reshape(P, BL * DC)
        )                                            # [P, BL*DC]
        if v_bf16 and not bf16:
            import ml_dtypes
            in_maps.append({"enc_t": enc_t, "we": we, "c": c,
                            "v": v.astype(ml_dtypes.bfloat16)})
            continue
        if bf16:
            import ml_dtypes
            enc_t = enc_t.astype(ml_dtypes.bfloat16)
            in_maps.append({
                "enc_t": enc_t,
                "we": we.astype(ml_dtypes.bfloat16),
                "c": c,
                "v": v.astype(ml_dtypes.bfloat16),
            })
        else:
            in_maps.append({"enc_t": enc_t, "we": we, "c": c, "v": v})
    return in_maps


def run_full(inputs: dict, trace: bool = False):
    """Run on 8 cores; returns (full_output [B,S], BassKernelResults)."""
    nc = _get_nc()
    in_maps = _prep_in_maps(**inputs)
    res = run_bass_kernel_spmd(
        nc, in_maps, list(range(N_CORES)), trace=trace
    )
    out = np.concatenate(
        [res.results[i]["out"] for i in range(N_CORES)], axis=0
    )
    return out, res


def kernel(**inputs) -> np.ndarray:
    out, _ = run_full(inputs)
    return out



# revision 2
# speedup vs baseline: 1.0541x; 1.0541x over previous
"""Bahdanau-style attention scores kernel for 8 TRN2 NeuronCores.

Reference math (B=64, S=2048, E=512, D=512):
    Wh = attn_W[:D]; We = attn_W[D:]
    h_proj = hidden @ Wh                                  # [B, D]
    e_proj[b,s,:] = enc[b,s,:] @ We                       # [B, S, D]
    energy = tanh(h_proj[:,None,:] + e_proj + attn_b)     # [B, S, D]
    scores = energy @ v_w                                 # [B, S]
    out = softmax(scores, axis=1)

Sharding: data-parallel over batch, 8 batches per core.
Host precomputes c = hidden @ Wh + attn_b (tiny: 33 MFLOP), converts
enc/We/v to bf16 (halves HBM traffic; absmax_rel ~4e-3, gate is 2e-2),
and transposes enc to [b, e, s] so the contraction axis e lands on SBUF
partitions.

Per-core kernel (b = 0..7 local batches), d-outer / s-inner:
  for d: 16 matmuls (4e x 4s) accumulate e_projT[d_chunk, s] into 2-bank
         PSUM tiles; consecutive matmuls share lhsT (LDW pressure /4)
  energyT = tanh(psum + c_b[d]) -> bf16, one ACT call per [128, 1024]
  scores matvecs v.T @ energyT packed 4-wide via tile_position col strips
  softmax tail on the 4 score rows (partitions 0/32/64/96), deferred one
  batch for overlap.
A few dummy warm-up matmuls at t=0 lift the PE HAM clock gate to 2.4 GHz
before the first real matmul's DMA dependencies land.
"""

import numpy as np

import concourse.bass as bass  # noqa: F401  (engine namespaces via nc)
import concourse.mybir as mybir
import concourse.tile as tile
from concourse import bacc, bass_isa
from concourse.bass_utils import run_bass_kernel_spmd

N_CORES = 8
B, S, E, D = 64, 2048, 512, 512
BL = B // N_CORES          # local batches per core
P = 128                    # partition tile
EC = E // P                # e chunks (4)
DC = D // P                # d chunks (4)
ST = 512                   # s tile (free dim per matmul; one PSUM bank f32)
SC = S // ST               # s tiles (4)

DT = mybir.dt.bfloat16     # matmul input dtype (enc, We, v, energy)

_COMPILED = None  # nc cache within the process


def _build(warmup=9, enc_bufs=3, psp_bufs=3, en_bufs=4):
    nc = bacc.Bacc(
        "TRN2", target_bir_lowering=False, debug=False, num_devices=N_CORES
    )
    f32 = mybir.dt.float32

    enc_ap = nc.dram_tensor("enc_t", [BL, E, S], DT, kind="ExternalInput").ap()
    we_ap = nc.dram_tensor("we", [E, D], DT, kind="ExternalInput").ap()
    c_ap = nc.dram_tensor("c", [P, BL * DC], f32, kind="ExternalInput").ap()
    v_ap = nc.dram_tensor("v", [P, DC], DT, kind="ExternalInput").ap()
    out_ap = nc.dram_tensor("out", [BL, S], f32, kind="ExternalOutput").ap()

    with tile.TileContext(nc) as tc:
        with (
            tc.tile_pool(name="singles", bufs=1) as singles,
            tc.tile_pool(name="encp", bufs=enc_bufs) as encp,
            tc.tile_pool(name="enp", bufs=en_bufs) as enp,
            tc.tile_pool(name="expp", bufs=2) as expp,
            tc.tile_pool(name="smallp", bufs=2) as smallp,
            tc.tile_pool(name="outp", bufs=2) as outp,
            tc.tile_pool(name="warmp", bufs=1) as warmp,
            tc.tile_pool(name="psp", bufs=psp_bufs, space="PSUM") as psp,
            tc.tile_pool(name="scp", bufs=2, space="PSUM") as scp,
        ):
            # ---- PE warm-up: run the HAM activity window hot before the
            # first real matmul's DMA deps arrive (dummy data, never read).
            wsb = warmp.tile([P, ST], DT)
            nc.vector.memset(wsb, 0.0)
            wps = scp.tile([P, ST], f32, name="warm_ps", tag="sc")
            for _ in range(warmup):
                nc.tensor.matmul(wps, lhsT=wsb[:, 0:P], rhs=wsb, start=True,
                                 stop=True)

            we_sb = singles.tile([P, EC, D], DT)
            c_sb = singles.tile([P, BL * DC], f32)
            v_sb = singles.tile([P, DC], DT)

            def emit_weight_dmas():
                # split per e-chunk so the first matmul's lhsT dep lands
                # after 128 KB instead of 512 KB
                we_src = we_ap.rearrange("(e p) d -> p e d", p=P)
                for e in range(EC):
                    nc.sync.dma_start(
                        out=we_sb[:, e : e + 1, :], in_=we_src[:, e : e + 1, :]
                    )
                nc.sync.dma_start(out=c_sb, in_=c_ap)
                nc.sync.dma_start(out=v_sb, in_=v_ap)

            pend = []   # deferred score matvecs, one d behind
            tails = []  # deferred per-batch softmax tails

            def flush_pend():
                for scores_t, s, d, en_t in pend:
                    nc.tensor.matmul(
                        scores_t[32 * s : 32 * s + 1, :],
                        lhsT=v_sb[:, d : d + 1],
                        rhs=en_t,
                        start=(d == 0),
                        stop=(d == DC - 1),
                        tile_position=(0, 32 * s),
                    )
                pend.clear()

            def flush_and_tail():
                flush_pend()
                if tails:
                    tails.pop(0)()

            for b in range(BL):
                enc_sb = encp.tile([P, EC, S], DT, name=f"enc_b{b}", tag="enc")
                enc_src = enc_ap[b].rearrange("(e p) s -> p e s", p=P)
                for s in range(SC):
                    nc.sync.dma_start(
                        out=enc_sb[:, :, s * ST : (s + 1) * ST],
                        in_=enc_src[:, :, s * ST : (s + 1) * ST],
                    )
                    if b == 0 and s == 0:
                        emit_weight_dmas()

                scores = scp.tile([P, ST], f32, name=f"scores_b{b}", tag="sc")

                for d in range(DC):
                    # two 2-bank psum tiles hold the 4 s-tiles of this d
                    ps_pair = [
                        psp.tile([P, 2, ST], f32, name=f"ps_b{b}d{d}h{h}",
                                 tag="ps")
                        for h in range(2)
                    ]
                    for e in range(EC):
                        for s in range(SC):
                            nc.tensor.matmul(
                                ps_pair[s // 2][:, s % 2, :],
                                lhsT=we_sb[:, e, d * P : (d + 1) * P],
                                rhs=enc_sb[:, e, s * ST : (s + 1) * ST],
                                start=(e == 0),
                                stop=(e == EC - 1),
                            )
                    flush_and_tail()
                    for h in range(2):
                        en = enp.tile([P, 2, ST], DT, name=f"en_b{b}d{d}h{h}",
                                      tag="en")
                        nc.scalar.activation(
                            en,
                            ps_pair[h],
                            mybir.ActivationFunctionType.Tanh,
                            bias=c_sb[:, b * DC + d : b * DC + d + 1],
                        )
                        for g in range(2):
                            s = 2 * h + g
                            pend.append((scores, s, d, en[:, g, :]))

                def make_tail(b=b, scores=scores):
                    def tail():
                        ssum4 = smallp.tile(
                            [P, 1], f32, name=f"ssum4_b{b}", tag="ssum4"
                        )
                        nc.vector.memset(ssum4, 0.0)
                        expw = expp.tile(
                            [P, ST], f32, name=f"expw_b{b}", tag="expw"
                        )
                        nc.vector.memset(expw, 0.0)
                        for s in range(SC):
                            nc.scalar.activation(
                                expw[32 * s : 32 * s + 1, :],
                                scores[32 * s : 32 * s + 1, :],
                                mybir.ActivationFunctionType.Exp,
                                accum_out=ssum4[32 * s : 32 * s + 1, 0:1],
                            )
                        tot = smallp.tile([P, 1], f32, name=f"tot_b{b}",
                                          tag="tot")
                        nc.gpsimd.partition_all_reduce(
                            tot, ssum4, 128, bass_isa.ReduceOp.add
                        )
                        rec = smallp.tile([P, 1], f32, name=f"rec_b{b}",
                                          tag="rec")
                        nc.vector.reciprocal(rec, tot)
                        outw = outp.tile(
                            [P, ST], f32, name=f"outw_b{b}", tag="outw"
                        )
                        nc.vector.tensor_scalar_mul(outw, expw, rec)
                        nc.sync.dma_start(
                            out=out_ap[b].rearrange("(r s) -> r s", r=SC),
                            in_=outw[0 : 32 * (SC - 1) + 1 : 32, :],
                        )
                    return tail

                tails.append(make_tail())

            flush_and_tail()
            flush_and_tail()

    nc.compile()
    return nc


def _get_nc():
    global _COMPILED
    if _COMPILED is None:
        _COMPILED = _build()
    return _COMPILED


def _prep_in_maps(hidden, encoder_outputs, attn_W, attn_b, v_w):
    import ml_dtypes

    hidden = np.asarray(hidden, dtype=np.float32)
    encoder_outputs = np.asarray(encoder_outputs, dtype=np.float32)
    attn_W = np.asarray(attn_W, dtype=np.float32)
    attn_b = np.asarray(attn_b, dtype=np.float32)
    v_w = np.asarray(v_w, dtype=np.float32)

    c_full = hidden @ attn_W[:D] + attn_b            # [B, D]
    we = np.ascontiguousarray(attn_W[D:]).astype(ml_dtypes.bfloat16)
    v = np.ascontiguousarray(v_w.reshape(DC, P).T).astype(ml_dtypes.bfloat16)

    in_maps = []
    for i in range(N_CORES):
        lo = i * BL
        enc_t = np.ascontiguousarray(
            encoder_outputs[:, lo : lo + BL, :].transpose(1, 2, 0)
        ).astype(ml_dtypes.bfloat16)                 # [BL, E, S]
        c_shard = c_full[lo : lo + BL]               # [BL, D]
        c = np.ascontiguousarray(
            c_shard.reshape(BL, DC, P).transpose(2, 0, 1).reshape(P, BL * DC)
        )                                            # [P, BL*DC]
        in_maps.append({"enc_t": enc_t, "we": we, "c": c, "v": v})
    return in_maps


def run_full(inputs: dict, trace: bool = False):
    """Run on 8 cores; returns (full_output [B,S], BassKernelResults)."""
    nc = _get_nc()
    in_maps = _prep_in_maps(**inputs)
    res = run_bass_kernel_spmd(
        nc, in_maps, list(range(N_CORES)), trace=trace
    )
    out = np.concatenate(
        [res.results[i]["out"] for i in range(N_CORES)], axis=0
    )
    return out, res


def kernel(**inputs) -> np.ndarray:
    out, _ = run_full(inputs)
    return out
